# revision 8
# baseline (speedup 1.0000x reference)
"""DOSACon loss on 8 Trainium2 NeuronCores (Bass/Tile, SPMD data-parallel).

Math: the [N,N] broadcast in the localization term is rank-1 separable --
  mean(dw * hw * (1-ciou)^g / (area+eps)) over [N,N]
    = (sum_i dw_i*hw_i*(1-ciou_i)^g) * (sum_j 1/(area_j+eps)) / N^2
so each core computes partial sums over its 1024-row shard of the N=8192
boxes.  The 100-pair contrastive term is gathered on host (pure data
movement) and evaluated on-device in a packed 9th column / pair lane.

v3 design:
 - arctan difference via atan(a)-atan(b) = atan((a-b)/(1+ab)): one [128,9]
   arctan, ratio-prep on Pool.  ACT runs ONLY Arctan and Sigmoid -- both
   resolve to the sigmoid_and_others table, so exactly one table load,
   hoisted into the preamble (Square/Identity/Copy resolve to
   exp_and_others first and would trigger a second 1.28us load mid-chain).
 - sqrt((1-ciou)) via float-domain exponent halving (bits/2 + K) plus one
   Newton step refined with reciprocal_approx_fast: ~1e-3 rel, tol is 2e-2.
 - 1-ciou is assembled as (1-iou) + penalty; ciou itself never
   materializes, and sigmoid reads it via scale=5 bias=-2.5 on 1-ciou.
 - per-partition partials are reduced across partitions ON-CHIP by a PE
   matmul with a ones vector -> [1,18] PSUM -> SBUF -> single-descriptor
   output DMA (a [128,x] output DMA needs 16 completion-semaphore
   increments which straggle 1.5-4us; one descriptor completes with one).
 - input DMA split across two DGE queues (sync: boxes, gpsimd: emb).
 - engine programs are order-pinned: the Tile scheduler otherwise hoists
   the aspect-ratio chain ahead of the iou spine and idles DVE ~1.7us.
"""

from contextlib import ExitStack

import numpy as np

N_CORES = 8
N = 8192
NS = N // N_CORES      # 1024 boxes per core
PPART = 128            # SBUF partitions
FREE = NS // PPART     # 8 shard columns
W = FREE + 1           # 9 = shard columns + 1 pair column
D = 256
NPAIR = 100

GAMMA = 2.5
ALPHA_D = 1.2
DELTA = 1.0
TAU = 0.3
LAMBDA_C = 0.5
EPS = 1e-7
SQRT_MAGIC = float(0x1FBD1DF5)   # float-domain sqrt-bits seed constant

_BUILT = None          # cached nc across calls
LAST_RESULT = None     # last BassKernelResults (for profiling in test.py)


def _build_nc():
    import concourse.bacc as bacc
    import concourse.mybir as mybir
    import concourse.tile as tile
    from concourse.tile import add_dep_helper

    dt = mybir.dt.float32
    du = mybir.dt.uint32
    A = mybir.AluOpType
    AF = mybir.ActivationFunctionType
    AX = mybir.AxisListType
    VS = 4.0 / np.pi ** 2

    nc = bacc.Bacc("TRN2", target_bir_lowering=False, debug=False,
                   num_devices=N_CORES)
    buf_d = nc.dram_tensor("buf", [PPART, 592], dt, kind="ExternalInput")
    out_d = nc.dram_tensor("out", [1, 18], dt, kind="ExternalOutput")

    with tile.TileContext(nc) as tc, ExitStack() as ctx:
        pool = ctx.enter_context(tc.tile_pool(name="p", bufs=1))
        ppool = ctx.enter_context(
            tc.tile_pool(name="ps", bufs=1, space="PSUM"))

        def T(n, tag, dtype=dt):
            return pool.tile([PPART, n], dtype, name=tag, tag=tag)

        V, S, G, PE = nc.vector, nc.scalar, nc.gpsimd, nc.tensor

        def pin(chain):
            """Keep each engine's instruction stream in written order."""
            for a, b in zip(chain, chain[1:]):
                add_dep_helper(b.ins, a.ins, sync=False, reason="pin")

        bufA = T(80, "bufA")
        bufB = T(512, "bufB")
        # boxes via the sync DGE queue, embeddings via gpsimd's: the two
        # descriptor generations overlap instead of serializing
        nc.sync.dma_start(bufA[:], buf_d.ap()[:, 0:80])
        G.dma_start(bufB[:], buf_d.ap()[:, 80:592])

        P2 = bufA[:, 0:36]      # px|py|tx|ty blocks of 9
        WH = bufA[:, 36:72]     # pw|ph|tw|th blocks of 9
        dn = bufA[:, 72:80]
        ei = bufB[:, 0:256]
        ej = bufB[:, 256:512]
        whr = WH.rearrange("p (a b) -> p a b", b=W)
        w_in = whr[:, 0::2, :]   # pw|tw  [128,2,9]
        h_in = whr[:, 1::2, :]   # ph|th  [128,2,9]
        wh_lo = whr[:, 0:2, :]   # pw|ph
        wh_hi = whr[:, 2:4, :]   # tw|th
        pw = WH[:, 0:9]
        ph = WH[:, 9:18]
        tw = WH[:, 18:27]
        th = WH[:, 27:36]

        def r2(ap):
            return ap.rearrange("p (a b) -> p a b", b=W)

        # constants (no data deps; Pool runs them during the DMA window)
        bm25 = T(1, "bm25")
        G.memset(bm25[:], -2.5)
        ones = T(1, "ones")
        G.memset(ones[:], 1.0)

        fin = T(18, "fin")              # scr | ia | masked hinge | pad
        G.memset(fin[:, 2 * FREE + 1:18], 0.0)

        # ================= Pool program =================
        # order matters: aden gates the arctan chain, the c-chain and dv2
        # gate the alpha chain; density/distance terms are needed later
        gc = []
        ar = T(18, "ar")                # pw*ph | tw*th
        gc.append(G.tensor_tensor(r2(ar[:]), w_in, h_in, A.mult))
        u0 = T(W, "u0")                 # a1 + a2  (union + inter)
        gc.append(G.tensor_tensor(u0[:], ar[:, 0:W], ar[:, W:2 * W], A.add))
        # arctan-difference prep: r = (tw*ph - pw*th) / (ph*th + pw*tw)
        q1 = T(W, "q1")
        gc.append(G.tensor_tensor(q1[:], pw, th, A.mult))
        q2 = T(W, "q2")
        gc.append(G.tensor_tensor(q2[:], tw, ph, A.mult))
        anum = T(W, "anum")
        gc.append(G.tensor_tensor(anum[:], q2[:], q1[:], A.subtract))
        qwh = T(18, "qwh")              # pw*tw | ph*th
        gc.append(G.tensor_tensor(r2(qwh[:]), wh_lo, wh_hi, A.mult))
        aden = T(W, "aden")
        gc.append(G.tensor_tensor(aden[:], qwh[:, 0:W], qwh[:, W:2 * W],
                                  A.add))
        ad = T(FREE, "ad")              # target area + 1e-7
        gc.append(G.tensor_scalar(ad[:], ar[:, W:W + FREE], 1e-7, None,
                                  A.add))

        # ================= DVE spine =================
        vc = []
        lohi = T(72, "lohi")            # lo1|lo2 | hi1|hi2 blocks of 18
        lo = lohi[:, 0:36]
        hi = lohi[:, 36:72]
        vc.append(V.scalar_tensor_tensor(lo, WH, -0.5, P2, A.mult, A.add))
        vc.append(V.scalar_tensor_tensor(hi, WH, 0.5, P2, A.mult, A.add))
        lhr = lohi[:].rearrange("p (a b) -> p a b", b=18)  # [128,4,18]
        mx = T(36, "mx")                # mlo | c0
        vc.append(V.tensor_tensor(mx[:].rearrange("p (a b) -> p a b", b=18),
                                  lhr[:, 0::2, :], lhr[:, 1::2, :], A.max))
        mn = T(36, "mn")                # c1 | mhi
        vc.append(V.tensor_tensor(mn[:].rearrange("p (a b) -> p a b", b=18),
                                  lhr[:, 0::2, :], lhr[:, 1::2, :], A.min))
        mlo = mx[:, 0:18]
        c0 = mx[:, 18:36]
        c1 = mn[:, 0:18]
        mhi = mn[:, 18:36]
        iwh = T(18, "iwh")
        vc.append(V.tensor_tensor(iwh[:], mhi, mlo, A.subtract))
        iwr = T(18, "iwr")
        vc.append(V.tensor_scalar_max(iwr[:], iwh[:], 0.0))
        inter = T(W, "inter")
        vc.append(V.tensor_tensor(inter[:], iwr[:, 0:W], iwr[:, W:2 * W],
                                  A.mult))
        u2 = T(W, "u2")                 # union = u0 - inter
        vc.append(V.scalar_tensor_tensor(u2[:], inter[:], -1.0, u0[:],
                                         A.mult, A.add))
        ru = T(W, "ru")
        vc.append(V.reciprocal_approx_fast(ru[:], u2[:]))
        iou = T(W, "iou")
        vc.append(V.tensor_tensor(iou[:], inter[:], ru[:], A.mult))
        rden = T(W, "rden")
        vc.append(V.reciprocal_approx_fast(rden[:], aden[:]))
        rz = T(W, "rz")
        vc.append(V.tensor_tensor(rz[:], anum[:], rden[:], A.mult))
        # off-critical helpers while ACT runs arctan
        iou1m = T(W, "iou1m")           # iou - (1+eps)
        vc.append(V.tensor_scalar(iou1m[:], iou[:], 1.0, -(1.0 + EPS),
                                  A.mult, A.add))
        oiou = T(W, "oiou")             # 1 - iou
        vc.append(V.tensor_scalar(oiou[:], iou[:], -1.0, 1.0, A.mult, A.add))
        # embeddings: diff, then fused square+row-sum (custom DVE op)
        diff = T(D, "diff")
        vc.append(V.tensor_tensor(diff[:], ei, ej, A.subtract))
        omd = T(W, "omd")               # 1-ciou cols 0:8, pair |e|^2 col 8
        sqs = T(D, "sqs")
        vc.append(V.affine_mul_reduce(sqs[:], omd[:, FREE:W], diff[:],
                                      diff[:], 1.0, 0.0))
        ia = fin[:, FREE:2 * FREE]
        vc.append(V.reciprocal_approx_fast(ia, ad[:]))

        # ================= ACT: arctan =================
        ats = T(W, "ats")
        at_i = S.activation(ats[:], rz[:], AF.Arctan)

        # ================= Pool mid =================
        cwh = T(18, "cwh")
        gc.append(G.tensor_tensor(cwh[:], c0, c1, A.subtract))
        csq = T(18, "csq")
        gc.append(G.tensor_tensor(csq[:], cwh[:], cwh[:], A.mult))
        c2 = T(W, "c2")
        gc.append(G.tensor_tensor(c2[:], csq[:, 0:W], csq[:, W:2 * W],
                                  A.add))
        dv2 = T(W, "dv2")               # atan_diff^2; v = VS*dv2
        gc.append(G.tensor_tensor(dv2[:], ats[:], ats[:], A.mult))
        vv = T(W, "vv")                 # dv2^2
        gc.append(G.tensor_tensor(vv[:], dv2[:], dv2[:], A.mult))
        dwt = T(FREE, "dwt")            # 1 + 1.2*density
        gc.append(G.tensor_scalar(dwt[:], dn, ALPHA_D, 1.0, A.mult, A.add))
        dxy = T(18, "dxy")
        gc.append(G.tensor_tensor(dxy[:], P2[:, 18:36], P2[:, 0:18],
                                  A.subtract))
        dsq = T(18, "dsq")
        gc.append(G.tensor_tensor(dsq[:], dxy[:], dxy[:], A.mult))
        rho2 = T(W, "rho2")
        gc.append(G.tensor_tensor(rho2[:], dsq[:, 0:W], dsq[:, W:2 * W],
                                  A.add))
        mask = T(1, "mask")             # pair_iou > tau
        gc.append(G.tensor_scalar(mask[:], iou[:, FREE:W], TAU, None,
                                  A.is_gt))

        # ================= DVE alpha chain -> omd =================
        d1 = T(W, "d1")                 # v - iou + 1 + eps
        vc.append(V.scalar_tensor_tensor(d1[:], dv2[:], VS, iou1m[:],
                                         A.mult, A.subtract))
        rd = T(W, "rd")
        vc.append(V.reciprocal_approx_fast(rd[:], d1[:]))
        va = T(W, "va")                 # v^2/d1 = v*alpha
        vc.append(V.scalar_tensor_tensor(va[:], vv[:], VS * VS, rd[:],
                                         A.mult, A.mult))
        rc2 = T(W, "rc2")
        vc.append(V.reciprocal_approx_fast(rc2[:], c2[:]))
        rr = T(W, "rr")
        vc.append(V.tensor_tensor(rr[:], rho2[:], rc2[:], A.mult))
        pen = T(W, "pen")
        vc.append(V.tensor_tensor(pen[:], rr[:], va[:], A.add))
        vc.append(V.tensor_tensor(omd[:, 0:FREE], oiou[:, 0:FREE],
                                  pen[:, 0:FREE], A.add))


        # ================= ACT: sigmoid(5*(omd-0.5)) =================
        hwt = T(FREE, "hwt")
        sg_i = S.activation(hwt[:], omd[:, 0:FREE], AF.Sigmoid, scale=5.0,
                            bias=bm25[:])
        add_dep_helper(sg_i.ins, at_i.ins, sync=False, reason="pin")

        # ================= Pool tail: sqrt seed + masks =================
        fbits = T(W, "fbits")
        gc.append(G.tensor_copy(fbits[:], omd[:].bitcast(du)))  # u32->f32
        gbits = T(W, "gbits")
        gc.append(G.tensor_scalar(gbits[:], fbits[:], 0.5, SQRT_MAGIC,
                                  A.mult, A.add))
        y0u = T(W, "y0u", du)
        gc.append(G.tensor_copy(y0u[:], gbits[:]))              # f32->u32
        y0 = y0u[:].bitcast(dt)
        sq = T(FREE, "sq")              # (1-ciou)^2
        gc.append(G.tensor_tensor(sq[:], omd[:, 0:FREE], omd[:, 0:FREE],
                                  A.mult))
        mask2 = T(1, "mask2")           # |e|^2 < 1 (else hinge is 0)
        gc.append(G.tensor_scalar(mask2[:], omd[:, FREE:W], 1.0, None,
                                  A.is_lt))
        maskb = T(1, "maskb")
        gc.append(G.tensor_tensor(maskb[:], mask[:], mask2[:], A.mult))
        s1p = T(1, "s1p")               # 1 + |e|^2
        gc.append(G.tensor_scalar(s1p[:], omd[:, FREE:W], 1.0, None, A.add))

        # ================= DVE tail =================
        hrec = T(W, "hrec")
        vc.append(V.reciprocal_approx_fast(hrec[:], y0))
        ha = T(W, "ha")
        vc.append(V.scalar_tensor_tensor(ha[:], omd[:], 0.5, hrec[:],
                                         A.mult, A.mult))
        st = T(W, "st")                 # sqrt(1-ciou) | pair dist
        vc.append(V.scalar_tensor_tensor(st[:], y0, 0.5, ha[:],
                                         A.mult, A.add))
        m1 = T(FREE, "m1")
        vc.append(V.tensor_tensor(m1[:], dwt[:], hwt[:], A.mult))
        # hinge = 1 + s - 2*sqrt(s) (valid where s<1; masked otherwise)
        hv = T(1, "hv")
        vc.append(V.scalar_tensor_tensor(hv[:], st[:, FREE:W], -2.0, s1p[:],
                                         A.mult, A.add))
        vc.append(V.tensor_tensor(fin[:, 2 * FREE:2 * FREE + 1], maskb[:],
                                  hv[:], A.mult))

        # Pool: p25 = (1-ciou)^2.5, then DVE: scr into fin
        p25 = T(FREE, "p25")
        gc.append(G.tensor_tensor(p25[:], sq[:], st[:, 0:FREE], A.mult))
        vc.append(V.tensor_tensor(fin[:, 0:FREE], m1[:], p25[:], A.mult))

        pin(gc)
        pin(vc)

        # ================= PE reduce + copy out =================
        psum = ppool.tile([1, 18], dt, name="psum", tag="psum")
        PE.matmul(psum[:], ones[:], fin[:], start=True, stop=True)
        red = pool.tile([1, 18], dt, name="red", tag="red")
        V.tensor_copy(red[:], psum[:])

        nc.sync.dma_start(out_d.ap(), red[:], single_packet=True)

    nc.compile()
    return nc


def _get_nc():
    global _BUILT
    if _BUILT is None:
        _BUILT = _build_nc()
    return _BUILT


def _pack_inputs(pred_boxes, target_boxes, embeddings, density_map, indices):
    pred = np.ascontiguousarray(pred_boxes, dtype=np.float32)
    targ = np.ascontiguousarray(target_boxes, dtype=np.float32)
    emb = np.ascontiguousarray(embeddings, dtype=np.float32)
    dens = np.ascontiguousarray(density_map, dtype=np.float32)
    idx = np.asarray(indices).astype(np.int64)

    i0, i1 = idx[:, 0], idx[:, 1]
    bi = np.ones((PPART, 4), np.float32)
    bj = np.ones((PPART, 4), np.float32)
    bj[:, 0] = 10.0            # pad rows: far box -> pair_iou 0 -> mask 0
    bi[:NPAIR] = pred[i0]
    bj[:NPAIR] = pred[i1]
    ei = np.zeros((PPART, D), np.float32)
    ej = np.zeros((PPART, D), np.float32)
    ei[:NPAIR] = emb[i0]
    ej[:NPAIR] = emb[i1]

    in_maps = []
    for c in range(N_CORES):
        s = slice(c * NS, (c + 1) * NS)
        pbs = pred[s].reshape(PPART, FREE, 4)
        tbs = targ[s].reshape(PPART, FREE, 4)
        buf = np.empty((PPART, 592), np.float32)
        # P2 blocks: px py tx ty ; WH blocks: pw ph tw th
        for k, (src, comp) in enumerate(
                [(pbs, 0), (pbs, 1), (tbs, 0), (tbs, 1),
                 (pbs, 2), (pbs, 3), (tbs, 2), (tbs, 3)]):
            pair = (bi if src is pbs else bj)[:, comp]
            buf[:, k * W:k * W + FREE] = src[:, :, comp]
            buf[:, k * W + FREE] = pair
        buf[:, 72:80] = dens[s].reshape(PPART, FREE)
        buf[:, 80:336] = ei
        buf[:, 336:592] = ej
        in_maps.append({"buf": buf})
    return in_maps


def kernel(pred_boxes, target_boxes, embeddings, density_map, indices):
    global LAST_RESULT
    import time as _time

    from concourse.bass_utils import run_bass_kernel_spmd

    nc = _get_nc()
    in_maps = _pack_inputs(pred_boxes, target_boxes, embeddings,
                           density_map, indices)
    for attempt in range(3):
        try:
            res = run_bass_kernel_spmd(nc, in_maps,
                                       core_ids=list(range(N_CORES)))
            break
        except Exception:
            # a crashed earlier run can leave a core wedged
            # (NRT_EXEC_UNIT_UNRECOVERABLE); it clears on retry
            if attempt == 2:
                raise
            _time.sleep(2.0)
    LAST_RESULT = res

    outs = np.stack([res.results[c]["out"][0] for c in range(N_CORES)])
    s_a = float(np.sum(outs[:, 0:FREE], dtype=np.float64))
    s_b = float(np.sum(outs[:, FREE:2 * FREE], dtype=np.float64))
    contrast = float(outs[0, 2 * FREE])
    loss = s_a * s_b / (N * N) + LAMBDA_C * contrast / (NPAIR + 1e-7)
    return np.asarray(np.float32(loss))


# revision 9
# speedup vs baseline: 1.1178x; 1.1178x over previous
"""DOSACon loss on 8 Trainium2 NeuronCores (Bass/Tile, SPMD data-parallel).

Math: the [N,N] broadcast in the localization term is rank-1 separable --
  mean(dw * hw * (1-ciou)^g / (area+eps)) over [N,N]
    = (sum_i dw_i*hw_i*(1-ciou_i)^g) * (sum_j 1/(area_j+eps)) / N^2
so each core computes partial sums over its 1024-row shard of the N=8192
boxes.  The 100-pair contrastive term is gathered on host (pure data
movement) and evaluated on-device in a packed 9th column / pair lane.

v3 design:
 - arctan difference via atan(a)-atan(b) = atan((a-b)/(1+ab)): one [128,9]
   arctan, ratio-prep on Pool.  ACT runs ONLY Arctan and Sigmoid -- both
   resolve to the sigmoid_and_others table, so exactly one table load,
   hoisted into the preamble (Square/Identity/Copy resolve to
   exp_and_others first and would trigger a second 1.28us load mid-chain).
 - sqrt((1-ciou)) via float-domain exponent halving (bits/2 + K) plus one
   Newton step refined with reciprocal_approx_fast: ~1e-3 rel, tol is 2e-2.
 - 1-ciou is assembled as (1-iou) + penalty; ciou itself never
   materializes, and sigmoid reads it via scale=5 bias=-2.5 on 1-ciou.
 - per-partition partials are reduced across partitions ON-CHIP by a PE
   matmul with a ones vector -> [1,18] PSUM -> SBUF -> single-descriptor
   output DMA (a [128,x] output DMA needs 16 completion-semaphore
   increments which straggle 1.5-4us; one descriptor completes with one).
 - input DMA split across two DGE queues (sync: boxes, gpsimd: emb).
 - engine programs are order-pinned: the Tile scheduler otherwise hoists
   the aspect-ratio chain ahead of the iou spine and idles DVE ~1.7us.
"""

from contextlib import ExitStack

import numpy as np

N_CORES = 8
N = 8192
NS = N // N_CORES      # 1024 boxes per core
PPART = 128            # SBUF partitions
FREE = NS // PPART     # 8 shard columns
W = FREE + 1           # 9 = shard columns + 1 pair column
D = 256
NPAIR = 100

GAMMA = 2.5
ALPHA_D = 1.2
DELTA = 1.0
TAU = 0.3
LAMBDA_C = 0.5
EPS = 1e-7
SQRT_MAGIC = float(0x1FBD1DF5)   # float-domain sqrt-bits seed constant

_BUILT = None          # cached nc across calls
LAST_RESULT = None     # last BassKernelResults (for profiling in test.py)


def _build_nc():
    import concourse.bacc as bacc
    import concourse.mybir as mybir
    import concourse.tile as tile
    from concourse.tile import add_dep_helper

    dt = mybir.dt.float32
    du = mybir.dt.uint32
    A = mybir.AluOpType
    AF = mybir.ActivationFunctionType
    AX = mybir.AxisListType
    VS = 4.0 / np.pi ** 2

    nc = bacc.Bacc("TRN2", target_bir_lowering=False, debug=False,
                   num_devices=N_CORES)
    buf_d = nc.dram_tensor("buf", [PPART, 592], dt, kind="ExternalInput")
    out_d = nc.dram_tensor("out", [1, 18], dt, kind="ExternalOutput")

    with tile.TileContext(nc) as tc, ExitStack() as ctx:
        pool = ctx.enter_context(tc.tile_pool(name="p", bufs=1))
        ppool = ctx.enter_context(
            tc.tile_pool(name="ps", bufs=1, space="PSUM"))

        def T(n, tag, dtype=dt):
            return pool.tile([PPART, n], dtype, name=tag, tag=tag)

        V, S, G, PE = nc.vector, nc.scalar, nc.gpsimd, nc.tensor

        def pin(chain):
            """Keep each engine's instruction stream in written order."""
            for a, b in zip(chain, chain[1:]):
                add_dep_helper(b.ins, a.ins, sync=False, reason="pin")

        bufA = T(80, "bufA")
        bufB = T(512, "bufB")
        # boxes via the sync DGE queue, embeddings via gpsimd's: the two
        # descriptor generations overlap instead of serializing
        nc.sync.dma_start(bufA[:], buf_d.ap()[:, 0:80])
        G.dma_start(bufB[:], buf_d.ap()[:, 80:592])

        P2 = bufA[:, 0:36]      # px|py|tx|ty blocks of 9
        WH = bufA[:, 36:72]     # pw|ph|tw|th blocks of 9
        dn = bufA[:, 72:80]
        ei = bufB[:, 0:256]
        ej = bufB[:, 256:512]
        whr = WH.rearrange("p (a b) -> p a b", b=W)
        w_in = whr[:, 0::2, :]   # pw|tw  [128,2,9]
        h_in = whr[:, 1::2, :]   # ph|th  [128,2,9]
        wh_lo = whr[:, 0:2, :]   # pw|ph
        wh_hi = whr[:, 2:4, :]   # tw|th
        pw = WH[:, 0:9]
        ph = WH[:, 9:18]
        tw = WH[:, 18:27]
        th = WH[:, 27:36]

        def r2(ap):
            return ap.rearrange("p (a b) -> p a b", b=W)

        # constants (no data deps; Pool runs them during the DMA window)
        bm25 = T(1, "bm25")
        G.memset(bm25[:], -2.5)
        ones = T(1, "ones")
        G.memset(ones[:], 1.0)

        fin = T(18, "fin")              # scr | ia | masked hinge | pad
        G.memset(fin[:, 2 * FREE + 1:18], 0.0)

        # ================= Pool program =================
        # order matters: aden gates the arctan chain, the c-chain and dv2
        # gate the alpha chain; density/distance terms are needed later
        gc = []
        ar = T(18, "ar")                # pw*ph | tw*th
        gc.append(G.tensor_tensor(r2(ar[:]), w_in, h_in, A.mult))
        u0 = T(W, "u0")                 # a1 + a2  (union + inter)
        gc.append(G.tensor_tensor(u0[:], ar[:, 0:W], ar[:, W:2 * W], A.add))
        # arctan-difference prep: r = (tw*ph - pw*th) / (ph*th + pw*tw)
        q1 = T(W, "q1")
        gc.append(G.tensor_tensor(q1[:], pw, th, A.mult))
        q2 = T(W, "q2")
        gc.append(G.tensor_tensor(q2[:], tw, ph, A.mult))
        anum = T(W, "anum")
        gc.append(G.tensor_tensor(anum[:], q2[:], q1[:], A.subtract))
        qwh = T(18, "qwh")              # pw*tw | ph*th
        gc.append(G.tensor_tensor(r2(qwh[:]), wh_lo, wh_hi, A.mult))
        aden = T(W, "aden")
        gc.append(G.tensor_tensor(aden[:], qwh[:, 0:W], qwh[:, W:2 * W],
                                  A.add))
        ad = T(FREE, "ad")              # target area + 1e-7
        gc.append(G.tensor_scalar(ad[:], ar[:, W:W + FREE], 1e-7, None,
                                  A.add))

        # ================= DVE spine =================
        vc = []
        lohi = T(72, "lohi")            # lo1|lo2 | hi1|hi2 blocks of 18
        lo = lohi[:, 0:36]
        hi = lohi[:, 36:72]
        vc.append(V.scalar_tensor_tensor(lo, WH, -0.5, P2, A.mult, A.add))
        vc.append(V.scalar_tensor_tensor(hi, WH, 0.5, P2, A.mult, A.add))
        lhr = lohi[:].rearrange("p (a b) -> p a b", b=18)  # [128,4,18]
        mx = T(36, "mx")                # mlo | c0
        vc.append(V.tensor_tensor(mx[:].rearrange("p (a b) -> p a b", b=18),
                                  lhr[:, 0::2, :], lhr[:, 1::2, :], A.max))
        mn = T(36, "mn")                # c1 | mhi
        vc.append(V.tensor_tensor(mn[:].rearrange("p (a b) -> p a b", b=18),
                                  lhr[:, 0::2, :], lhr[:, 1::2, :], A.min))
        mlo = mx[:, 0:18]
        c0 = mx[:, 18:36]
        c1 = mn[:, 0:18]
        mhi = mn[:, 18:36]
        iwh = T(18, "iwh")
        vc.append(V.tensor_tensor(iwh[:], mhi, mlo, A.subtract))
        iwr = T(18, "iwr")
        vc.append(V.tensor_scalar_max(iwr[:], iwh[:], 0.0))
        inter = T(W, "inter")
        vc.append(V.tensor_tensor(inter[:], iwr[:, 0:W], iwr[:, W:2 * W],
                                  A.mult))
        u2 = T(W, "u2")                 # union = u0 - inter
        vc.append(V.scalar_tensor_tensor(u2[:], inter[:], -1.0, u0[:],
                                         A.mult, A.add))
        ru = T(W, "ru")
        vc.append(V.reciprocal_approx_fast(ru[:], u2[:]))
        iou = T(W, "iou")
        vc.append(V.tensor_tensor(iou[:], inter[:], ru[:], A.mult))
        rden = T(W, "rden")
        vc.append(V.reciprocal_approx_fast(rden[:], aden[:]))
        rz = T(W, "rz")
        vc.append(V.tensor_tensor(rz[:], anum[:], rden[:], A.mult))

        # ---- ACT: arctan of the ratio difference (table-0 only) ----
        ats = T(W, "ats")
        at_i = S.activation(ats[:], rz[:], AF.Arctan)

        # ================= Pool mid =================
        # c-chain first (rc2 is needed in the arctan window), then rho2,
        # then dv2/vv the moment arctan lands, then the embeddings diff
        cwh = T(18, "cwh")
        gc.append(G.tensor_tensor(cwh[:], c0, c1, A.subtract))
        csq = T(18, "csq")
        gc.append(G.tensor_tensor(csq[:], cwh[:], cwh[:], A.mult))
        c2 = T(W, "c2")
        gc.append(G.tensor_tensor(c2[:], csq[:, 0:W], csq[:, W:2 * W],
                                  A.add))
        dxy = T(18, "dxy")
        gc.append(G.tensor_tensor(dxy[:], P2[:, 18:36], P2[:, 0:18],
                                  A.subtract))
        dsq = T(18, "dsq")
        gc.append(G.tensor_tensor(dsq[:], dxy[:], dxy[:], A.mult))
        rho2 = T(W, "rho2")
        gc.append(G.tensor_tensor(rho2[:], dsq[:, 0:W], dsq[:, W:2 * W],
                                  A.add))
        dv2 = T(W, "dv2")               # atan_diff^2; v = VS*dv2
        gc.append(G.tensor_tensor(dv2[:], ats[:], ats[:], A.mult))
        vv = T(W, "vv")                 # dv2^2
        gc.append(G.tensor_tensor(vv[:], dv2[:], dv2[:], A.mult))
        diff = T(D, "diff")
        gc.append(G.tensor_tensor(diff[:], ei, ej, A.subtract))
        mask = T(1, "mask")             # pair_iou > tau
        gc.append(G.tensor_scalar(mask[:], iou[:, FREE:W], TAU, None,
                                  A.is_gt))

        # ================= DVE: arctan-window work + alpha chain ========
        iou1m = T(W, "iou1m")           # iou - (1+eps)
        vc.append(V.tensor_scalar(iou1m[:], iou[:], 1.0, -(1.0 + EPS),
                                  A.mult, A.add))
        ia = fin[:, FREE:2 * FREE]
        vc.append(V.reciprocal_approx_fast(ia, ad[:]))
        rc2 = T(W, "rc2")
        vc.append(V.reciprocal_approx_fast(rc2[:], c2[:]))
        rr = T(W, "rr")
        vc.append(V.tensor_tensor(rr[:], rho2[:], rc2[:], A.mult))
        roi2 = T(W, "roi2")             # rr + 1 + eps - iou
        vc.append(V.tensor_tensor(roi2[:], rr[:], iou1m[:], A.subtract))
        d1 = T(W, "d1")                 # v - iou + 1 + eps
        vc.append(V.scalar_tensor_tensor(d1[:], dv2[:], VS, iou1m[:],
                                         A.mult, A.subtract))
        rd = T(W, "rd")
        vc.append(V.reciprocal_approx_fast(rd[:], d1[:]))
        va = T(W, "va")                 # v^2/d1 = v*alpha
        vc.append(V.scalar_tensor_tensor(va[:], vv[:], VS * VS, rd[:],
                                         A.mult, A.mult))
        omd = T(W, "omd")               # 1-ciou cols 0:8, pair |e|^2 col 8
        vc.append(V.tensor_tensor(omd[:, 0:FREE], roi2[:, 0:FREE],
                                  va[:, 0:FREE], A.add))
        sqs = T(D, "sqs")
        vc.append(V.affine_mul_reduce(sqs[:], omd[:, FREE:W], diff[:],
                                      diff[:], 1.0, 0.0))

        # ================= ACT: sigmoid(5*(omd-0.5)) =================
        hwt = T(FREE, "hwt")
        sg_i = S.activation(hwt[:], omd[:, 0:FREE], AF.Sigmoid, scale=5.0,
                            bias=bm25[:])
        add_dep_helper(sg_i.ins, at_i.ins, sync=False, reason="pin")

        # ================= Pool tail: sqrt seed + masks =================
        fbits = T(W, "fbits")
        gc.append(G.tensor_copy(fbits[:], omd[:].bitcast(du)))  # u32->f32
        gbits = T(W, "gbits")
        gc.append(G.tensor_scalar(gbits[:], fbits[:], 0.5, SQRT_MAGIC,
                                  A.mult, A.add))
        y0u = T(W, "y0u", du)
        gc.append(G.tensor_copy(y0u[:], gbits[:]))              # f32->u32
        y0 = y0u[:].bitcast(dt)
        sq = T(FREE, "sq")              # (1-ciou)^2
        gc.append(G.tensor_tensor(sq[:], omd[:, 0:FREE], omd[:, 0:FREE],
                                  A.mult))
        mask2 = T(1, "mask2")           # |e|^2 < 1 (else hinge is 0)
        gc.append(G.tensor_scalar(mask2[:], omd[:, FREE:W], 1.0, None,
                                  A.is_lt))
        maskb = T(1, "maskb")
        gc.append(G.tensor_tensor(maskb[:], mask[:], mask2[:], A.mult))
        s1p = T(1, "s1p")               # 1 + |e|^2
        gc.append(G.tensor_scalar(s1p[:], omd[:, FREE:W], 1.0, None, A.add))
        dwt = T(FREE, "dwt")            # 1 + 1.2*density
        gc.append(G.tensor_scalar(dwt[:], dn, ALPHA_D, 1.0, A.mult, A.add))

        # ================= DVE tail =================
        m1 = T(FREE, "m1")
        vc.append(V.tensor_tensor(m1[:], dwt[:], hwt[:], A.mult))
        hrec = T(W, "hrec")
        vc.append(V.reciprocal_approx_fast(hrec[:], y0))
        ha = T(W, "ha")
        vc.append(V.scalar_tensor_tensor(ha[:], omd[:], 0.5, hrec[:],
                                         A.mult, A.mult))
        st = T(W, "st")                 # sqrt(1-ciou) | pair dist
        vc.append(V.scalar_tensor_tensor(st[:], y0, 0.5, ha[:],
                                         A.mult, A.add))
        # hinge = 1 + s - 2*sqrt(s) (valid where s<1; masked otherwise)
        hv = T(1, "hv")
        vc.append(V.scalar_tensor_tensor(hv[:], st[:, FREE:W], -2.0, s1p[:],
                                         A.mult, A.add))
        vc.append(V.tensor_tensor(fin[:, 2 * FREE:2 * FREE + 1], maskb[:],
                                  hv[:], A.mult))

        # Pool: p25 = (1-ciou)^2.5, then DVE: scr into fin
        p25 = T(FREE, "p25")
        gc.append(G.tensor_tensor(p25[:], sq[:], st[:, 0:FREE], A.mult))
        vc.append(V.tensor_tensor(fin[:, 0:FREE], m1[:], p25[:], A.mult))

        pin(gc)
        pin(vc)

        # ================= PE reduce + copy out =================
        psum = ppool.tile([1, 18], dt, name="psum", tag="psum")
        PE.matmul(psum[:], ones[:], fin[:], start=True, stop=True)
        red = pool.tile([1, 18], dt, name="red", tag="red")
        V.tensor_copy(red[:], psum[:])

        nc.sync.dma_start(out_d.ap(), red[:], single_packet=True)

    nc.compile()
    return nc


def _get_nc():
    global _BUILT
    if _BUILT is None:
        _BUILT = _build_nc()
    return _BUILT


def _pack_inputs(pred_boxes, target_boxes, embeddings, density_map, indices):
    pred = np.ascontiguousarray(pred_boxes, dtype=np.float32)
    targ = np.ascontiguousarray(target_boxes, dtype=np.float32)
    emb = np.ascontiguousarray(embeddings, dtype=np.float32)
    dens = np.ascontiguousarray(density_map, dtype=np.float32)
    idx = np.asarray(indices).astype(np.int64)

    i0, i1 = idx[:, 0], idx[:, 1]
    bi = np.ones((PPART, 4), np.float32)
    bj = np.ones((PPART, 4), np.float32)
    bj[:, 0] = 10.0            # pad rows: far box -> pair_iou 0 -> mask 0
    bi[:NPAIR] = pred[i0]
    bj[:NPAIR] = pred[i1]
    ei = np.zeros((PPART, D), np.float32)
    ej = np.zeros((PPART, D), np.float32)
    ei[:NPAIR] = emb[i0]
    ej[:NPAIR] = emb[i1]

    in_maps = []
    for c in range(N_CORES):
        s = slice(c * NS, (c + 1) * NS)
        pbs = pred[s].reshape(PPART, FREE, 4)
        tbs = targ[s].reshape(PPART, FREE, 4)
        buf = np.empty((PPART, 592), np.float32)
        # P2 blocks: px py tx ty ; WH blocks: pw ph tw th
        for k, (src, comp) in enumerate(
                [(pbs, 0), (pbs, 1), (tbs, 0), (tbs, 1),
                 (pbs, 2), (pbs, 3), (tbs, 2), (tbs, 3)]):
            pair = (bi if src is pbs else bj)[:, comp]
            buf[:, k * W:k * W + FREE] = src[:, :, comp]
            buf[:, k * W + FREE] = pair
        buf[:, 72:80] = dens[s].reshape(PPART, FREE)
        buf[:, 80:336] = ei
        buf[:, 336:592] = ej
        in_maps.append({"buf": buf})
    return in_maps


def kernel(pred_boxes, target_boxes, embeddings, density_map, indices):
    global LAST_RESULT
    import time as _time

    from concourse.bass_utils import run_bass_kernel_spmd

    nc = _get_nc()
    in_maps = _pack_inputs(pred_boxes, target_boxes, embeddings,
                           density_map, indices)
    for attempt in range(3):
        try:
            res = run_bass_kernel_spmd(nc, in_maps,
                                       core_ids=list(range(N_CORES)))
            break
        except Exception:
            # a crashed earlier run can leave a core wedged
            # (NRT_EXEC_UNIT_UNRECOVERABLE); it clears on retry
            if attempt == 2:
                raise
            _time.sleep(2.0)
    LAST_RESULT = res

    outs = np.stack([res.results[c]["out"][0] for c in range(N_CORES)])
    s_a = float(np.sum(outs[:, 0:FREE], dtype=np.float64))
    s_b = float(np.sum(outs[:, FREE:2 * FREE], dtype=np.float64))
    contrast = float(outs[0, 2 * FREE])
    loss = s_a * s_b / (N * N) + LAMBDA_C * contrast / (NPAIR + 1e-7)
    return np.asarray(np.float32(loss))


# revision 10
# speedup vs baseline: 1.1640x; 1.0414x over previous
"""DOSACon loss on 8 Trainium2 NeuronCores (Bass/Tile, SPMD data-parallel).

Math: the [N,N] broadcast in the localization term is rank-1 separable --
  mean(dw * hw * (1-ciou)^g / (area+eps)) over [N,N]
    = (sum_i dw_i*hw_i*(1-ciou_i)^g) * (sum_j 1/(area_j+eps)) / N^2
so each core computes partial sums over its 1024-row shard of the N=8192
boxes.  The 100-pair contrastive term is gathered on host (pure data
movement) and evaluated on-device in a packed 9th column / pair lane.

v3 design:
 - arctan difference via atan(a)-atan(b) = atan((a-b)/(1+ab)): one [128,9]
   arctan, ratio-prep on Pool.  ACT runs ONLY Arctan and Sigmoid -- both
   resolve to the sigmoid_and_others table, so exactly one table load,
   hoisted into the preamble (Square/Identity/Copy resolve to
   exp_and_others first and would trigger a second 1.28us load mid-chain).
 - sqrt((1-ciou)) via float-domain exponent halving (bits/2 + K) plus one
   Newton step refined with reciprocal_approx_fast: ~1e-3 rel, tol is 2e-2.
 - 1-ciou is assembled as (1-iou) + penalty; ciou itself never
   materializes, and sigmoid reads it via scale=5 bias=-2.5 on 1-ciou.
 - per-partition partials are reduced across partitions ON-CHIP by a PE
   matmul with a ones vector -> [1,18] PSUM -> SBUF -> single-descriptor
   output DMA (a [128,x] output DMA needs 16 completion-semaphore
   increments which straggle 1.5-4us; one descriptor completes with one).
 - input DMA split across two DGE queues (sync: boxes, gpsimd: emb).
 - engine programs are order-pinned: the Tile scheduler otherwise hoists
   the aspect-ratio chain ahead of the iou spine and idles DVE ~1.7us.
"""

from contextlib import ExitStack

import numpy as np

N_CORES = 8
N = 8192
NS = N // N_CORES      # 1024 boxes per core
PPART = 128            # SBUF partitions
FREE = NS // PPART     # 8 shard columns
W = FREE + 1           # 9 = shard columns + 1 pair column
D = 256
NPAIR = 100

GAMMA = 2.5
ALPHA_D = 1.2
DELTA = 1.0
TAU = 0.3
LAMBDA_C = 0.5
EPS = 1e-7
SQRT_MAGIC = float(0x1FBD1DF5)   # float-domain sqrt-bits seed constant

_BUILT = None          # cached nc across calls
LAST_RESULT = None     # last BassKernelResults (for profiling in test.py)


def _build_nc():
    import concourse.bacc as bacc
    import concourse.mybir as mybir
    import concourse.tile as tile
    from concourse.tile import add_dep_helper

    dt = mybir.dt.float32
    du = mybir.dt.uint32
    A = mybir.AluOpType
    AF = mybir.ActivationFunctionType
    AX = mybir.AxisListType
    VS = 4.0 / np.pi ** 2

    nc = bacc.Bacc("TRN2", target_bir_lowering=False, debug=False,
                   num_devices=N_CORES)
    buf_d = nc.dram_tensor("buf", [PPART, 592], dt, kind="ExternalInput")
    out_d = nc.dram_tensor("out", [1, 18], dt, kind="ExternalOutput")

    with tile.TileContext(nc) as tc, ExitStack() as ctx:
        pool = ctx.enter_context(tc.tile_pool(name="p", bufs=1))
        ppool = ctx.enter_context(
            tc.tile_pool(name="ps", bufs=1, space="PSUM"))

        def T(n, tag, dtype=dt):
            return pool.tile([PPART, n], dtype, name=tag, tag=tag)

        V, S, G, PE = nc.vector, nc.scalar, nc.gpsimd, nc.tensor

        def pin(chain):
            """Keep each engine's instruction stream in written order."""
            for a, b in zip(chain, chain[1:]):
                add_dep_helper(b.ins, a.ins, sync=False, reason="pin")

        bufA = T(80, "bufA")
        bufB = T(512, "bufB")
        # boxes via the sync DGE queue, embeddings via gpsimd's: the two
        # descriptor generations overlap instead of serializing
        nc.sync.dma_start(bufA[:], buf_d.ap()[:, 0:80])
        G.dma_start(bufB[:], buf_d.ap()[:, 80:592])

        P2 = bufA[:, 0:36]      # px|py|tx|ty blocks of 9
        WH = bufA[:, 36:72]     # pw|ph|tw|th blocks of 9
        dn = bufA[:, 72:80]
        ei = bufB[:, 0:256]
        ej = bufB[:, 256:512]
        whr = WH.rearrange("p (a b) -> p a b", b=W)
        w_in = whr[:, 0::2, :]   # pw|tw  [128,2,9]
        h_in = whr[:, 1::2, :]   # ph|th  [128,2,9]
        wh_lo = whr[:, 0:2, :]   # pw|ph
        wh_hi = whr[:, 2:4, :]   # tw|th
        pw = WH[:, 0:9]
        ph = WH[:, 9:18]
        tw = WH[:, 18:27]
        th = WH[:, 27:36]

        def r2(ap):
            return ap.rearrange("p (a b) -> p a b", b=W)

        # constants (no data deps; Pool runs them during the DMA window)
        bm25 = T(1, "bm25")
        G.memset(bm25[:], -2.5)
        ones = T(1, "ones")
        G.memset(ones[:], 1.0)

        fin = T(18, "fin")              # scr | ia | masked hinge | pad
        G.memset(fin[:, 2 * FREE + 1:18], 0.0)

        # ================= Pool program =================
        # order matters: aden gates the arctan chain, the c-chain and dv2
        # gate the alpha chain; density/distance terms are needed later
        gc = []
        ar = T(18, "ar")                # pw*ph | tw*th
        gc.append(G.tensor_tensor(r2(ar[:]), w_in, h_in, A.mult))
        u0 = T(W, "u0")                 # a1 + a2  (union + inter)
        gc.append(G.tensor_tensor(u0[:], ar[:, 0:W], ar[:, W:2 * W], A.add))
        # arctan-difference prep: r = (tw*ph - pw*th) / (ph*th + pw*tw)
        q1 = T(W, "q1")
        gc.append(G.tensor_tensor(q1[:], pw, th, A.mult))
        q2 = T(W, "q2")
        gc.append(G.tensor_tensor(q2[:], tw, ph, A.mult))
        anum = T(W, "anum")
        gc.append(G.tensor_tensor(anum[:], q2[:], q1[:], A.subtract))
        qwh = T(18, "qwh")              # pw*tw | ph*th
        gc.append(G.tensor_tensor(r2(qwh[:]), wh_lo, wh_hi, A.mult))
        aden = T(W, "aden")
        gc.append(G.tensor_tensor(aden[:], qwh[:, 0:W], qwh[:, W:2 * W],
                                  A.add))
        ad = T(FREE, "ad")              # target area + 1e-7
        gc.append(G.tensor_scalar(ad[:], ar[:, W:W + FREE], 1e-7, None,
                                  A.add))
        dwt = T(FREE, "dwt")            # 1 + 1.2*density
        gc.append(G.tensor_scalar(dwt[:], dn, ALPHA_D, 1.0, A.mult, A.add))

        # ================= DVE spine =================
        vc = []
        lohi = T(72, "lohi")            # lo1|lo2 | hi1|hi2 blocks of 18
        lo = lohi[:, 0:36]
        hi = lohi[:, 36:72]
        vc.append(V.scalar_tensor_tensor(lo, WH, -0.5, P2, A.mult, A.add))
        vc.append(V.scalar_tensor_tensor(hi, WH, 0.5, P2, A.mult, A.add))
        lhr = lohi[:].rearrange("p (a b) -> p a b", b=18)  # [128,4,18]
        mx = T(36, "mx")                # mlo | c0
        vc.append(V.tensor_tensor(mx[:].rearrange("p (a b) -> p a b", b=18),
                                  lhr[:, 0::2, :], lhr[:, 1::2, :], A.max))
        mn = T(36, "mn")                # c1 | mhi
        vc.append(V.tensor_tensor(mn[:].rearrange("p (a b) -> p a b", b=18),
                                  lhr[:, 0::2, :], lhr[:, 1::2, :], A.min))
        mlo = mx[:, 0:18]
        c0 = mx[:, 18:36]
        c1 = mn[:, 0:18]
        mhi = mn[:, 18:36]
        iwh = T(18, "iwh")
        vc.append(V.tensor_tensor(iwh[:], mhi, mlo, A.subtract))
        iwr = T(18, "iwr")
        vc.append(V.tensor_scalar_max(iwr[:], iwh[:], 0.0))
        inter = T(W, "inter")
        vc.append(V.tensor_tensor(inter[:], iwr[:, 0:W], iwr[:, W:2 * W],
                                  A.mult))
        u2 = T(W, "u2")                 # union = u0 - inter
        vc.append(V.scalar_tensor_tensor(u2[:], inter[:], -1.0, u0[:],
                                         A.mult, A.add))
        ru = T(W, "ru")
        vc.append(V.reciprocal_approx_fast(ru[:], u2[:]))
        iou = T(W, "iou")
        vc.append(V.tensor_tensor(iou[:], inter[:], ru[:], A.mult))
        rden = T(W, "rden")
        vc.append(V.reciprocal_approx_fast(rden[:], aden[:]))
        rz = T(W, "rz")
        vc.append(V.tensor_tensor(rz[:], anum[:], rden[:], A.mult))

        # ---- ACT: arctan of the ratio difference (table-0 only) ----
        ats = T(W, "ats")
        at_i = S.activation(ats[:], rz[:], AF.Arctan)

        # ================= Pool mid =================
        # c-chain first (rc2 is needed in the arctan window), then rho2,
        # then dv2/vv the moment arctan lands, then the embeddings diff
        cwh = T(18, "cwh")
        gc.append(G.tensor_tensor(cwh[:], c0, c1, A.subtract))
        csq = T(18, "csq")
        gc.append(G.tensor_tensor(csq[:], cwh[:], cwh[:], A.mult))
        c2 = T(W, "c2")
        gc.append(G.tensor_tensor(c2[:], csq[:, 0:W], csq[:, W:2 * W],
                                  A.add))
        dv2 = T(W, "dv2")               # atan_diff^2; v = VS*dv2
        gc.append(G.tensor_tensor(dv2[:], ats[:], ats[:], A.mult))
        vv = T(W, "vv")                 # dv2^2
        gc.append(G.tensor_tensor(vv[:], dv2[:], dv2[:], A.mult))
        diff = T(D, "diff")
        gc.append(G.tensor_tensor(diff[:], ei, ej, A.subtract))
        mask = T(1, "mask")             # pair_iou > tau
        gc.append(G.tensor_scalar(mask[:], iou[:, FREE:W], TAU, None,
                                  A.is_gt))

        # ================= DVE: arctan-window work + alpha chain ========
        iou1m = T(W, "iou1m")           # iou - (1+eps)
        vc.append(V.tensor_scalar(iou1m[:], iou[:], 1.0, -(1.0 + EPS),
                                  A.mult, A.add))
        ia = fin[:, FREE:2 * FREE]
        vc.append(V.reciprocal_approx_fast(ia, ad[:]))
        dxy = T(18, "dxy")
        vc.append(V.tensor_tensor(dxy[:], P2[:, 18:36], P2[:, 0:18],
                                  A.subtract))
        dsq = T(18, "dsq")
        vc.append(V.tensor_tensor(dsq[:], dxy[:], dxy[:], A.mult))
        rho2 = T(W, "rho2")
        vc.append(V.tensor_tensor(rho2[:], dsq[:, 0:W], dsq[:, W:2 * W],
                                  A.add))
        rc2 = T(W, "rc2")
        vc.append(V.reciprocal_approx_fast(rc2[:], c2[:]))
        rr = T(W, "rr")
        vc.append(V.tensor_tensor(rr[:], rho2[:], rc2[:], A.mult))
        roi2 = T(W, "roi2")             # rr + 1 + eps - iou
        vc.append(V.tensor_tensor(roi2[:], rr[:], iou1m[:], A.subtract))
        d1 = T(W, "d1")                 # v - iou + 1 + eps
        vc.append(V.scalar_tensor_tensor(d1[:], dv2[:], VS, iou1m[:],
                                         A.mult, A.subtract))
        rd = T(W, "rd")
        vc.append(V.reciprocal_approx_fast(rd[:], d1[:]))
        va = T(W, "va")                 # v^2/d1 = v*alpha
        vc.append(V.scalar_tensor_tensor(va[:], vv[:], VS * VS, rd[:],
                                         A.mult, A.mult))
        omd = T(W, "omd")               # 1-ciou cols 0:8, pair |e|^2 col 8
        sqs = T(D, "sqs")
        vc.append(V.affine_mul_reduce(sqs[:], omd[:, FREE:W], diff[:],
                                      diff[:], 1.0, 0.0))
        vc.append(V.tensor_tensor(omd[:, 0:FREE], roi2[:, 0:FREE],
                                  va[:, 0:FREE], A.add))

        # ================= ACT: sigmoid(5*(omd-0.5)) =================
        hwt = T(FREE, "hwt")
        sg_i = S.activation(hwt[:], omd[:, 0:FREE], AF.Sigmoid, scale=5.0,
                            bias=bm25[:])
        add_dep_helper(sg_i.ins, at_i.ins, sync=False, reason="pin")

        # ================= Pool tail: sqrt seed + masks =================
        fbits = T(W, "fbits")
        gc.append(G.tensor_copy(fbits[:], omd[:].bitcast(du)))  # u32->f32
        gbits = T(W, "gbits")
        gc.append(G.tensor_scalar(gbits[:], fbits[:], 0.5, SQRT_MAGIC,
                                  A.mult, A.add))
        y0u = T(W, "y0u", du)
        gc.append(G.tensor_copy(y0u[:], gbits[:]))              # f32->u32
        y0 = y0u[:].bitcast(dt)
        sq = T(FREE, "sq")              # (1-ciou)^2
        gc.append(G.tensor_tensor(sq[:], omd[:, 0:FREE], omd[:, 0:FREE],
                                  A.mult))
        mask2 = T(1, "mask2")           # |e|^2 < 1 (else hinge is 0)
        gc.append(G.tensor_scalar(mask2[:], omd[:, FREE:W], 1.0, None,
                                  A.is_lt))
        maskb = T(1, "maskb")
        gc.append(G.tensor_tensor(maskb[:], mask[:], mask2[:], A.mult))
        s1p = T(1, "s1p")               # 1 + |e|^2
        gc.append(G.tensor_scalar(s1p[:], omd[:, FREE:W], 1.0, None, A.add))

        # ================= DVE tail =================
        hrec = T(W, "hrec")
        vc.append(V.reciprocal_approx_fast(hrec[:], y0))
        ha = T(W, "ha")
        vc.append(V.scalar_tensor_tensor(ha[:], omd[:], 0.5, hrec[:],
                                         A.mult, A.mult))
        st = T(W, "st")                 # sqrt(1-ciou) | pair dist
        vc.append(V.scalar_tensor_tensor(st[:], y0, 0.5, ha[:],
                                         A.mult, A.add))
        m1 = T(FREE, "m1")
        vc.append(V.tensor_tensor(m1[:], dwt[:], hwt[:], A.mult))
        # hinge = 1 + s - 2*sqrt(s) (valid where s<1; masked otherwise)
        hv = T(1, "hv")
        vc.append(V.scalar_tensor_tensor(hv[:], st[:, FREE:W], -2.0, s1p[:],
                                         A.mult, A.add))
        vc.append(V.tensor_tensor(fin[:, 2 * FREE:2 * FREE + 1], maskb[:],
                                  hv[:], A.mult))

        # Pool: p25 = (1-ciou)^2.5, then DVE: scr into fin
        p25 = T(FREE, "p25")
        gc.append(G.tensor_tensor(p25[:], sq[:], st[:, 0:FREE], A.mult))
        vc.append(V.tensor_tensor(fin[:, 0:FREE], m1[:], p25[:], A.mult))

        pin(gc)
        pin(vc)

        # ================= PE reduce + copy out =================
        psum = ppool.tile([1, 18], dt, name="psum", tag="psum")
        PE.matmul(psum[:], ones[:], fin[:], start=True, stop=True)
        red = pool.tile([1, 18], dt, name="red", tag="red")
        V.tensor_copy(red[:], psum[:])

        nc.sync.dma_start(out_d.ap(), red[:], single_packet=True)

    nc.compile()
    return nc


def _get_nc():
    global _BUILT
    if _BUILT is None:
        _BUILT = _build_nc()
    return _BUILT


def _pack_inputs(pred_boxes, target_boxes, embeddings, density_map, indices):
    pred = np.ascontiguousarray(pred_boxes, dtype=np.float32)
    targ = np.ascontiguousarray(target_boxes, dtype=np.float32)
    emb = np.ascontiguousarray(embeddings, dtype=np.float32)
    dens = np.ascontiguousarray(density_map, dtype=np.float32)
    idx = np.asarray(indices).astype(np.int64)

    i0, i1 = idx[:, 0], idx[:, 1]
    bi = np.ones((PPART, 4), np.float32)
    bj = np.ones((PPART, 4), np.float32)
    bj[:, 0] = 10.0            # pad rows: far box -> pair_iou 0 -> mask 0
    bi[:NPAIR] = pred[i0]
    bj[:NPAIR] = pred[i1]
    ei = np.zeros((PPART, D), np.float32)
    ej = np.zeros((PPART, D), np.float32)
    ei[:NPAIR] = emb[i0]
    ej[:NPAIR] = emb[i1]

    in_maps = []
    for c in range(N_CORES):
        s = slice(c * NS, (c + 1) * NS)
        pbs = pred[s].reshape(PPART, FREE, 4)
        tbs = targ[s].reshape(PPART, FREE, 4)
        buf = np.empty((PPART, 592), np.float32)
        # P2 blocks: px py tx ty ; WH blocks: pw ph tw th
        for k, (src, comp) in enumerate(
                [(pbs, 0), (pbs, 1), (tbs, 0), (tbs, 1),
                 (pbs, 2), (pbs, 3), (tbs, 2), (tbs, 3)]):
            pair = (bi if src is pbs else bj)[:, comp]
            buf[:, k * W:k * W + FREE] = src[:, :, comp]
            buf[:, k * W + FREE] = pair
        buf[:, 72:80] = dens[s].reshape(PPART, FREE)
        buf[:, 80:336] = ei
        buf[:, 336:592] = ej
        in_maps.append({"buf": buf})
    return in_maps


def kernel(pred_boxes, target_boxes, embeddings, density_map, indices):
    global LAST_RESULT
    import time as _time

    from concourse.bass_utils import run_bass_kernel_spmd

    nc = _get_nc()
    in_maps = _pack_inputs(pred_boxes, target_boxes, embeddings,
                           density_map, indices)
    for attempt in range(3):
        try:
            res = run_bass_kernel_spmd(nc, in_maps,
                                       core_ids=list(range(N_CORES)))
            break
        except Exception:
            # a crashed earlier run can leave a core wedged
            # (NRT_EXEC_UNIT_UNRECOVERABLE); it clears on retry
            if attempt == 2:
                raise
            _time.sleep(2.0)
    LAST_RESULT = res

    outs = np.stack([res.results[c]["out"][0] for c in range(N_CORES)])
    s_a = float(np.sum(outs[:, 0:FREE], dtype=np.float64))
    s_b = float(np.sum(outs[:, FREE:2 * FREE], dtype=np.float64))
    contrast = float(outs[0, 2 * FREE])
    loss = s_a * s_b / (N * N) + LAMBDA_C * contrast / (NPAIR + 1e-7)
    return np.asarray(np.float32(loss))
